# revision 16
# baseline (speedup 1.0000x reference)
"""DRC layer (dynamic range compressor) Trainium2 Bass kernel.

Problem: per batch row, y = x * 10^(-y_L/20) * 10^(mk/20) where y_L is a
branching one-pole smoother (attack/release) over the static gain curve
x_L computed in dB domain.  The smoother y[n] = a*y[n-1] + (1-a)*x_L[n]
with a in {alpha_A, alpha_R} chosen by (x_L[n] > y[n-1]) is solved by a
fixed-point iteration: guess y -> branch decisions -> the recurrence is
linear -> solve exactly with the hardware tensor_tensor_scan -> repeat.
In v = x_L - y space the recurrence is v[n] = a[n]*(v[n-1] - delta[n]),
delta[n] = x_L[n-1]-x_L[n], which is exactly one scan op
(state = (negdelta + state) * a).  Cross-chunk carries are solved exactly
each sweep with a tiny transposed scan over per-chunk affine maps
(A = prod a = exp(lR*L + (lA-lR)*sum d), f = v_end - A*v_init).
Schedule: 6 sweeps with one Aitken-extrapolated sweep (auto gamma from
boundary-delta ratios), then a final re-scan with exact carries.

Sharding: data-parallel, 2 batch rows per core x 8 cores.  Each core
packs its 2 rows as [128, 8192] (partitions 0-63 = row 0 in 64 chunks of
8192 samples, 64-127 = row 1).
"""
import sys
import numpy as np

try:
    from concourse import bass, bacc, mybir
except Exception:  # pragma: no cover
    for p in ("/opt/trn_rl_repo", "/root/.axon_site/_ro/trn_rl_repo"):
        if p not in sys.path:
            sys.path.insert(0, p)
    from concourse import bass, bacc, mybir

from concourse.bass_utils import run_bass_kernel_spmd
from concourse.tile import TileContext

f32 = np.float32
dt = mybir.dt
Op = mybir.AluOpType
Act = mybir.ActivationFunctionType

SR = f32(44100.0)
LOG9 = float(np.log(9.0))
CL = f32(20.0 / np.log(10.0))       # ln -> dB scale
RGAIN = f32(np.log(10.0) / 20.0)    # dB -> ln scale
P = 128                             # partitions
ROWS_PER_CORE = 2
N_CORES = 8
SCHED = "IIIEII"                    # I=sweep, E=extrapolated sweep
NCONST = 13


def host_consts(params):
    """params [R,6] float32 -> per-row constants [R, NCONST] float32.
    Mirrors the reference's float32 arithmetic for the alphas."""
    p = params.astype(f32)
    p = np.where(np.isnan(p), f32(0.0), p)
    p = np.where(p == 0, f32(1e-10), p)
    T = (-p[:, 0] * f32(60.0)).astype(f32)
    ratio = (p[:, 1] * f32(10.0)).astype(f32)
    attack = np.maximum((p[:, 2] / f32(10.0)).astype(f32), f32(1e-4))
    release = np.maximum((p[:, 3] * f32(3.0)).astype(f32), f32(0.005))
    W = (p[:, 4] * f32(24.0)).astype(f32)
    mk = (p[:, 5] * f32(20.0)).astype(f32)
    aA = np.exp((f32(-LOG9) / (SR * attack)).astype(f32)).astype(f32)
    aR = np.exp((f32(-LOG9) / (SR * release)).astype(f32)).astype(f32)
    # derived (host f64 where it only affects our solver internals)
    lA = np.log(aA.astype(np.float64))
    lR = np.log(aR.astype(np.float64))
    c1 = (1.0 - 1.0 / ratio.astype(np.float64)).astype(f32)
    negc2 = (-1.0 / (8.0 * W.astype(np.float64) * ratio.astype(np.float64))).astype(f32)
    out = np.zeros((p.shape[0], NCONST), f32)
    out[:, 0] = -T                    # negT
    out[:, 1] = c1
    out[:, 2] = negc2
    out[:, 3] = W / 2                 # halfW
    out[:, 4] = -W / 2                # neghalfW
    out[:, 5] = W
    out[:, 6] = aR
    out[:, 7] = aA - aR               # dA
    out[:, 8] = 0.0                   # lRL: filled per-L at call site
    out[:, 9] = (lA - lR).astype(f32)  # dal
    out[:, 10] = (mk.astype(np.float64) * np.log(10.0) / 20.0).astype(f32)  # expbias
    out[:, 11] = 1e-8                 # eps for log
    dA = (aA - aR).astype(np.float64)
    dA = np.where(dA == 0, 1e-30, dA)
    out[:, 12] = ((1.0 - aR.astype(np.float64)) / dA).astype(f32)  # dstar
    out_lR = lR.astype(f32)
    return out, out_lR


def build_program(L):
    """Build the SPMD Bass program for chunk length L (8192 for the real
    problem). Returns the compiled Bacc."""
    nc = bacc.Bacc("TRN2", target_bir_lowering=False, debug=False,
                   num_devices=N_CORES)
    x_in = nc.dram_tensor("x", (P, L), dt.float32, kind="ExternalInput")
    cst_in = nc.dram_tensor("cst", (P, NCONST), dt.float32, kind="ExternalInput")
    aux_in = nc.dram_tensor("aux", (4, P), dt.float32, kind="ExternalInput")
    ident_in = nc.dram_tensor("ident", (P, P), dt.float32, kind="ExternalInput")
    y_out = nc.dram_tensor("y", (P, L), dt.float32, kind="ExternalOutput")

    v = nc.vector
    s = nc.scalar
    g = nc.gpsimd
    te = nc.tensor

    NB = 4                      # col blocks for iter/post pipelining
    LB = L // NB
    NBP = 8                     # pre col blocks
    LBP = L // NBP

    with TileContext(nc) as tc:
        with (
            tc.tile_pool(name="big", bufs=1) as big,
            tc.tile_pool(name="sm", bufs=2) as sm,
            tc.tile_pool(name="smk", bufs=4) as smk,
            tc.tile_pool(name="ps", bufs=1, space="PSUM") as ps,
            tc.tile_pool(name="dram", bufs=1, space="DRAM") as dram,
        ):
            # ---- persistent small tiles
            cst = sm.tile([P, NCONST], dt.float32, tag="cst")
            nc.sync.dma_start(out=cst[:], in_=cst_in[:])
            maskt = sm.tile([1, P], dt.float32, tag="maskt")
            nc.sync.dma_start(out=maskt[:], in_=aux_in[0:1, :])
            mtt = sm.tile([2, P], dt.float32, tag="mtt")
            nc.sync.dma_start(out=mtt[:], in_=aux_in[1:3, :])
            onest = sm.tile([1, 1], dt.float32, tag="onest")
            nc.sync.dma_start(out=onest[:], in_=aux_in[3:4, 0:1])
            ident = sm.tile([P, P], dt.float32, tag="ident")
            nc.sync.dma_start(out=ident[:], in_=ident_in[:])
            startmask = maskt[0:1, :]    # [1,128]: 0 at chunk 0 and 64
            mt = mtt[0:2, :]             # [2,128] row-block indicator
            ones11 = onest[0:1, 0:1]     # [1,1] = 1.0

            def col(i):
                return cst[:, i:i + 1]

            # ---- big slots S1..S5 (32KB/partition each)
            S1 = big.tile([P, L], dt.float32, tag="S1")  # ND
            S2 = big.tile([P, L], dt.float32, tag="S2")  # x -> temps -> D
            S3 = big.tile([P, L], dt.float32, tag="S3")  # XL -> (spill) Vtmp
            S4 = big.tile([P, L], dt.float32, tag="S4")  # V
            S5 = big.tile([P, L], dt.float32, tag="S5")  # d/a
            ma32 = big.tile([P, L], dt.int32, tag="S4")  # PRE-only alias of S4
            spill = dram.tile([P, L], dt.float32, tag="spill")

            # ================= PRE: x -> x_L, D, ND (col-blocked) ========
            for b in range(NBP):
                sl = slice(b * LBP, (b + 1) * LBP)
                nc.sync.dma_start(out=S2[:, sl], in_=x_in[:, sl])
                # ACT chain: abs -> ln -> u -> (u+W)^2
                s.activation(S1[:, sl], S2[:, sl], Act.Abs, bias=0.0, scale=1.0)
                s.activation(S2[:, sl], S1[:, sl], Act.Ln, bias=col(11), scale=1.0)
                s.activation(S1[:, sl], S2[:, sl], Act.Identity, bias=col(0),
                             scale=float(CL))
                s.activation(S2[:, sl], S1[:, sl], Act.Square, bias=col(5), scale=1.0)
                # knee (ACT: -c2*(u+W)^2), au/masks/selects on DVE
                s.activation(S3[:, sl], S2[:, sl], Act.Identity, bias=0.0,
                             scale=col(2))
                v.tensor_scalar(out=S5[:, sl], in0=S1[:, sl], scalar1=col(1),
                                scalar2=None, op0=Op.mult)
                v.tensor_scalar(out=ma32[:, sl], in0=S1[:, sl], scalar1=col(3),
                                scalar2=None, op0=Op.is_gt)
                v.copy_predicated(S3[:, sl], ma32[:, sl], S5[:, sl])
                v.tensor_scalar(out=S5[:, sl], in0=S1[:, sl], scalar1=col(4),
                                scalar2=None, op0=Op.is_ge)
                v.tensor_tensor(out=S3[:, sl], in0=S3[:, sl], in1=S5[:, sl],
                                op=Op.mult)
                # S3[:, sl] = x_L block. delta into S2 (cols shifted by 1)
                lo = b * LBP
                hi = (b + 1) * LBP
                v.tensor_tensor(out=S2[:, max(lo, 1):hi],
                                in0=S3[:, max(lo, 1) - 1:hi - 1],
                                in1=S3[:, max(lo, 1):hi], op=Op.subtract)
                s.activation(S1[:, max(lo, 1):hi], S2[:, max(lo, 1):hi],
                             Act.Identity, bias=0.0, scale=-1.0)
                nc.sync.dma_start(out=spill[:, sl], in_=S3[:, sl])
            # cross-chunk delta col 0: prevlast[p] = x_L[p-1, L-1], rows reset 0
            pl = smk.tile([P, 1], dt.float32, tag="pl")
            v.memset(pl[:], 0.0)
            nc.sync.dma_start(out=pl[1:P, :], in_=S3[0:P - 1, L - 1:L])
            v.memset(pl[64:65, :], 0.0)
            v.memset(pl[0:1, :], 0.0)
            v.tensor_tensor(out=S2[:, 0:1], in0=pl[:], in1=S3[:, 0:1],
                            op=Op.subtract)
            v.tensor_scalar(out=S1[:, 0:1], in0=S2[:, 0:1], scalar1=-1.0,
                            scalar2=None, op0=Op.mult)

            # ================= iteration machinery =================
            def boundary_chain(V_t, vinit_used, sd, bias_ap):
                """next v_init col from this sweep's (A, f).
                logA = dal*sd + bias_ap (bias holds lRL [+ dal*d0])."""
                logA = smk.tile([P, 1], dt.float32, tag="logA")
                v.scalar_tensor_tensor(out=logA[:], in0=sd, scalar=col(9),
                                       in1=bias_ap, op0=Op.mult, op1=Op.add)
                A_c = smk.tile([P, 1], dt.float32, tag="A_c")
                s.activation(A_c[:], logA[:], Act.Exp, bias=0.0, scale=1.0)
                t1 = smk.tile([P, 1], dt.float32, tag="t1")
                if vinit_used is None:
                    v.memset(t1[:], 0.0)
                else:
                    v.tensor_tensor(out=t1[:], in0=A_c[:], in1=vinit_used,
                                    op=Op.mult)
                f_c = smk.tile([P, 1], dt.float32, tag="f_c")
                v.tensor_tensor(out=f_c[:], in0=V_t[:, L - 1:L], in1=t1[:],
                                op=Op.subtract)
                ap_p = ps.tile([1, P], dt.float32, tag="ap_p")
                te.transpose(ap_p[:], A_c[:], ident[:])
                a_row = smk.tile([1, P], dt.float32, tag="a_row")
                v.tensor_tensor(out=a_row[:], in0=ap_p[:], in1=startmask,
                                op=Op.mult)
                fp_p = ps.tile([1, P], dt.float32, tag="fp_p")
                te.transpose(fp_p[:], f_c[:], ident[:])
                f_row = smk.tile([1, P], dt.float32, tag="f_row")
                v.tensor_copy(f_row[:], fp_p[:])
                zr = smk.tile([1, P], dt.float32, tag="zr")
                v.tensor_tensor_scan(zr[:], a_row[:], f_row[:], 0.0,
                                     Op.mult, Op.add)
                zs = smk.tile([1, P], dt.float32, tag="zs")
                v.memset(zs[:], 0.0)
                v.tensor_copy(zs[0:1, 1:P], zr[0:1, 0:P - 1])
                v.tensor_tensor(out=zs[:], in0=zs[:], in1=startmask, op=Op.mult)
                vip = ps.tile([P, 1], dt.float32, tag="vip")
                te.transpose(vip[:], zs[:], ones11)
                vic = smk.tile([P, 1], dt.float32, tag="vic")
                v.tensor_copy(vic[:], vip[:])
                return vic, zs

            def row_broadcast(pair_row):
                pr = ps.tile([2, 1], dt.float32, tag="pr")
                te.transpose(pr[:], pair_row, ones11)
                prs = smk.tile([2, 1], dt.float32, tag="prs")
                v.tensor_copy(prs[:], pr[:])
                cb = ps.tile([P, 1], dt.float32, tag="cb")
                te.matmul(cb[:], mt, prs[:])
                out = smk.tile([P, 1], dt.float32, tag="bc")
                v.tensor_copy(out[:], cb[:])
                return out

            vinit = None
            vinit_prev = None
            zrow_hist = []
            V_cur = None

            for k, step in enumerate(SCHED):
                sd = smk.tile([P, 1], dt.float32, tag="sd")
                lbias = col(8)  # lRL
                if k == 0:
                    v.tensor_scalar(out=S5[:], in0=S2[:], scalar1=0.0,
                                    scalar2=None, op0=Op.is_lt)
                    s.activation(S5[:], S5[:], Act.Identity, bias=0.0,
                                 scale=1.0, accum_out=sd[:])
                    vinit_used = None
                else:
                    Vp = V_cur
                    vic_used = vinit
                    if step == "E":
                        z0, z1, z2 = zrow_hist[-1], zrow_hist[-2], zrow_hist[-3]
                        d1 = smk.tile([1, P], dt.float32, tag="d1")
                        v.tensor_tensor(out=d1[:], in0=z0[:], in1=z1[:], op=Op.subtract)
                        s.activation(d1[:], d1[:], Act.Abs, bias=0.0, scale=1.0)
                        d2 = smk.tile([1, P], dt.float32, tag="d2")
                        v.tensor_tensor(out=d2[:], in0=z1[:], in1=z2[:], op=Op.subtract)
                        s.activation(d2[:], d2[:], Act.Abs, bias=0.0, scale=1.0)
                        rs = smk.tile([1, 2], dt.float32, tag="rs")
                        rs2 = smk.tile([1, 2], dt.float32, tag="rs2")
                        half = P // 2
                        v.tensor_reduce(rs[0:1, 0:1], d1[0:1, 0:half], mybir.AxisListType.X, Op.add)
                        v.tensor_reduce(rs[0:1, 1:2], d1[0:1, half:P], mybir.AxisListType.X, Op.add)
                        v.tensor_reduce(rs2[0:1, 0:1], d2[0:1, 0:half], mybir.AxisListType.X, Op.add)
                        v.tensor_reduce(rs2[0:1, 1:2], d2[0:1, half:P], mybir.AxisListType.X, Op.add)
                        v.tensor_scalar(out=rs2[:], in0=rs2[:], scalar1=1e-30,
                                        scalar2=None, op0=Op.add)
                        rho = smk.tile([1, 2], dt.float32, tag="rho")
                        v.reciprocal(rs2[:], rs2[:])
                        v.tensor_tensor(out=rho[:], in0=rs[:], in1=rs2[:], op=Op.mult)
                        v.tensor_scalar(out=rho[:], in0=rho[:], scalar1=0.95,
                                        scalar2=None, op0=Op.min)
                        om = smk.tile([1, 2], dt.float32, tag="om")
                        v.tensor_scalar(out=om[:], in0=rho[:], scalar1=-1.0,
                                        scalar2=1.0, op0=Op.mult, op1=Op.add)
                        v.reciprocal(om[:], om[:])
                        gam = smk.tile([1, 2], dt.float32, tag="gam")
                        v.tensor_tensor(out=gam[:], in0=rho[:], in1=om[:], op=Op.mult)
                        gcol = row_broadcast(gam[:])
                        gp1 = smk.tile([P, 1], dt.float32, tag="gp1")
                        v.tensor_scalar(out=gp1[:], in0=gcol[:], scalar1=1.0,
                                        scalar2=None, op0=Op.add)
                        Vm1 = S4 if V_cur is S3 else S3
                        v.tensor_scalar(out=Vm1[:], in0=Vm1[:], scalar1=gcol[:],
                                        scalar2=None, op0=Op.mult)
                        v.scalar_tensor_tensor(out=Vm1[:], in0=Vp[:],
                                               scalar=gp1[:], in1=Vm1[:],
                                               op0=Op.mult, op1=Op.subtract)
                        Vp = Vm1
                        dv = smk.tile([P, 1], dt.float32, tag="dv")
                        v.tensor_tensor(out=dv[:], in0=vinit[:], in1=vinit_prev[:],
                                        op=Op.subtract)
                        vice = smk.tile([P, 1], dt.float32, tag="vice")
                        v.scalar_tensor_tensor(out=vice[:], in0=dv[:], scalar=gcol[:],
                                               in1=vinit[:], op0=Op.mult, op1=Op.add)
                        vic_used = vice
                    # blocked d with per-block partial sums (stt bypass/is_gt)
                    sdb = smk.tile([P, NB], dt.float32, tag="sdb")
                    v.tensor_tensor(out=S5[:, 0:1], in0=vic_used[:], in1=S2[:, 0:1],
                                    op=Op.is_gt)
                    for b in range(NB):
                        lo = b * LB
                        hi = (b + 1) * LB
                        l2 = max(lo, 1)
                        v.scalar_tensor_tensor(out=S5[:, l2:hi],
                                               in0=Vp[:, l2 - 1:hi - 1],
                                               scalar=1.0, in1=S2[:, l2:hi],
                                               op0=Op.bypass, op1=Op.is_gt,
                                               accum_out=sdb[:, b:b + 1])
                    v.tensor_reduce(sd[:], sdb[:], mybir.AxisListType.X, Op.add)
                    # fold col-0 decision into the logA bias
                    tl = smk.tile([P, 1], dt.float32, tag="tl")
                    v.tensor_scalar(out=tl[:], in0=S5[:, 0:1], scalar1=col(9),
                                    scalar2=col(8), op0=Op.mult, op1=Op.add)
                    lbias = tl[:]
                    vinit_used = vic_used
                # a = dA*d + aR (ACT, hidden under the scan chain), then
                # row-start a[.,0]=1, and chained block scans
                Vout = S3 if k == 2 else S4
                a_dst = S3 if k == 0 else S5
                v.tensor_copy(S5[0:1, 0:1], cst[0:1, 12:13])
                v.tensor_copy(S5[64:65, 0:1], cst[64:65, 12:13])
                for b in range(NB):
                    lo = b * LB
                    hi = (b + 1) * LB
                    s.activation(a_dst[:, lo:hi], S5[:, lo:hi], Act.Identity,
                                 bias=col(6), scale=col(7))
                    if b == 0:
                        init_ap = 0.0 if vinit_used is None else vinit_used[:]
                    else:
                        init_ap = Vout[:, lo - 1:lo]
                    v.tensor_tensor_scan(Vout[:, lo:hi], S1[:, lo:hi],
                                         a_dst[:, lo:hi], init_ap,
                                         Op.add, Op.mult)
                V_cur = Vout
                vinit_prev = vinit
                vic, zs = boundary_chain(
                    Vout, None if vinit_used is None else vinit_used[:], sd[:],
                    lbias)
                vinit = vic
                zrow_hist.append(zs)
                if len(zrow_hist) > 3:
                    zrow_hist.pop(0)

            # final exact re-scan (blocked) with POST chained per block
            nc.sync.dma_start(out=S3[:], in_=spill[:])
            nc.sync.dma_start(out=S2[:], in_=x_in[:])
            for b in range(NB):
                lo = b * LB
                hi = (b + 1) * LB
                sl = slice(lo, hi)
                init_ap = vinit[:] if b == 0 else S4[:, lo - 1:lo]
                v.tensor_tensor_scan(S4[:, sl], S1[:, sl], S5[:, sl], init_ap,
                                     Op.add, Op.mult)
                v.tensor_tensor(out=S5[:, sl], in0=S4[:, sl], in1=S3[:, sl],
                                op=Op.subtract)
                v.tensor_scalar(out=S5[:, sl], in0=S5[:, sl], scalar1=96.0,
                                scalar2=None, op0=Op.min)
                s.activation(S1[:, sl], S5[:, sl], Act.Exp, bias=col(10),
                             scale=float(RGAIN))
                v.tensor_tensor(out=S3[:, sl], in0=S1[:, sl], in1=S2[:, sl],
                                op=Op.mult)
                nc.sync.dma_start(out=y_out[:, sl], in_=S3[:, sl])

    nc.compile()
    return nc


_CACHE = {}
PROFILE = False
LAST_EXEC_NS = None
LAST_RESULTS = None


def _get_program(L):
    if L not in _CACHE:
        _CACHE[L] = build_program(L)
    return _CACHE[L]


def make_core_inputs(x, params, L):
    """Full x [B,N], params [B,6] -> list of per-core input dicts."""
    B, N = x.shape
    n_cores = B // ROWS_PER_CORE
    rows_per_core = ROWS_PER_CORE
    csts, lR = host_consts(params)
    csts[:, 8] = (lR.astype(np.float64) * L).astype(f32)
    aux = np.zeros((4, P), f32)
    aux[0, :] = 1.0
    aux[0, 0] = 0.0
    aux[0, 64] = 0.0
    aux[1, 0:64] = 1.0   # Mt row 0 -> partitions 0..63
    aux[2, 64:128] = 1.0
    aux[3, 0] = 1.0      # ones11
    ident = np.eye(P, dtype=f32)
    in_maps = []
    for c in range(n_cores):
        rows = slice(c * rows_per_core, (c + 1) * rows_per_core)
        xs = np.ascontiguousarray(x[rows]).reshape(P, L)
        cc = np.repeat(csts[rows], P // rows_per_core, axis=0)
        in_maps.append(dict(x=xs, cst=np.ascontiguousarray(cc),
                            aux=aux, ident=ident))
    return in_maps


def kernel(x, params):
    x = np.asarray(x)
    params = np.asarray(params, f32)
    B, N = x.shape
    L = (N * ROWS_PER_CORE) // P
    nc = _get_program(L)
    in_maps = make_core_inputs(np.asarray(x, f32), params, L)
    global LAST_EXEC_NS, LAST_RESULTS
    res = run_bass_kernel_spmd(nc, in_maps, list(range(B // ROWS_PER_CORE)),
                               trace=PROFILE)
    LAST_EXEC_NS = res.exec_time_ns
    LAST_RESULTS = res
    outs = [r["y"].reshape(ROWS_PER_CORE, N) for r in res.results]
    return np.concatenate(outs, axis=0).astype(x.dtype, copy=False)


# revision 17
# speedup vs baseline: 1.0404x; 1.0404x over previous
"""DRC layer (dynamic range compressor) Trainium2 Bass kernel.

Problem: per batch row, y = x * 10^(-y_L/20) * 10^(mk/20) where y_L is a
branching one-pole smoother (attack/release) over the static gain curve
x_L computed in dB domain.  The smoother y[n] = a*y[n-1] + (1-a)*x_L[n]
with a in {alpha_A, alpha_R} chosen by (x_L[n] > y[n-1]) is solved by a
fixed-point iteration: guess y -> branch decisions -> the recurrence is
linear -> solve exactly with the hardware tensor_tensor_scan -> repeat.
In v = x_L - y space the recurrence is v[n] = a[n]*(v[n-1] - delta[n]),
delta[n] = x_L[n-1]-x_L[n], which is exactly one scan op
(state = (negdelta + state) * a).  Cross-chunk carries are solved exactly
each sweep with a tiny transposed scan over per-chunk affine maps
(A = prod a = exp(lR*L + (lA-lR)*sum d), f = v_end - A*v_init).
Schedule: 6 sweeps with one Aitken-extrapolated sweep (auto gamma from
boundary-delta ratios), then a final re-scan with exact carries.

Sharding: data-parallel, 2 batch rows per core x 8 cores.  Each core
packs its 2 rows as [128, 8192] (partitions 0-63 = row 0 in 64 chunks of
8192 samples, 64-127 = row 1).
"""
import sys
import numpy as np

try:
    from concourse import bass, bacc, mybir
except Exception:  # pragma: no cover
    for p in ("/opt/trn_rl_repo", "/root/.axon_site/_ro/trn_rl_repo"):
        if p not in sys.path:
            sys.path.insert(0, p)
    from concourse import bass, bacc, mybir

from concourse.bass_utils import run_bass_kernel_spmd
from concourse.tile import TileContext

f32 = np.float32
dt = mybir.dt
Op = mybir.AluOpType
Act = mybir.ActivationFunctionType

SR = f32(44100.0)
LOG9 = float(np.log(9.0))
CL = f32(20.0 / np.log(10.0))       # ln -> dB scale
RGAIN = f32(np.log(10.0) / 20.0)    # dB -> ln scale
P = 128                             # partitions
ROWS_PER_CORE = 2
N_CORES = 8
SCHED = "IIIEII"                    # I=sweep, E=extrapolated sweep
NCONST = 13


def host_consts(params):
    """params [R,6] float32 -> per-row constants [R, NCONST] float32.
    Mirrors the reference's float32 arithmetic for the alphas."""
    p = params.astype(f32)
    p = np.where(np.isnan(p), f32(0.0), p)
    p = np.where(p == 0, f32(1e-10), p)
    T = (-p[:, 0] * f32(60.0)).astype(f32)
    ratio = (p[:, 1] * f32(10.0)).astype(f32)
    attack = np.maximum((p[:, 2] / f32(10.0)).astype(f32), f32(1e-4))
    release = np.maximum((p[:, 3] * f32(3.0)).astype(f32), f32(0.005))
    W = (p[:, 4] * f32(24.0)).astype(f32)
    mk = (p[:, 5] * f32(20.0)).astype(f32)
    aA = np.exp((f32(-LOG9) / (SR * attack)).astype(f32)).astype(f32)
    aR = np.exp((f32(-LOG9) / (SR * release)).astype(f32)).astype(f32)
    # derived (host f64 where it only affects our solver internals)
    lA = np.log(aA.astype(np.float64))
    lR = np.log(aR.astype(np.float64))
    c1 = (1.0 - 1.0 / ratio.astype(np.float64)).astype(f32)
    negc2 = (-1.0 / (8.0 * W.astype(np.float64) * ratio.astype(np.float64))).astype(f32)
    CL64 = np.float64(20.0 / np.log(10.0))
    T64 = T.astype(np.float64); W64 = W.astype(np.float64)
    out = np.zeros((p.shape[0], NCONST), f32)
    out[:, 0] = (-c1.astype(np.float64) * T64).astype(f32)   # negc1T
    out[:, 1] = (c1.astype(np.float64) * CL64).astype(f32)   # c1CL
    out[:, 2] = negc2
    out[:, 3] = ((W64 / 2 + T64) / CL64).astype(f32)         # thr_above (on ln)
    out[:, 4] = ((T64 - W64 / 2) / CL64).astype(f32)         # thr_below (on ln)
    out[:, 5] = (W64 - T64).astype(f32)                      # W - T (square bias)
    out[:, 6] = aR
    out[:, 7] = aA - aR               # dA
    out[:, 8] = 0.0                   # lRL: filled per-L at call site
    out[:, 9] = (lA - lR).astype(f32)  # dal
    out[:, 10] = (mk.astype(np.float64) * np.log(10.0) / 20.0).astype(f32)  # expbias
    out[:, 11] = 1e-8                 # eps for log
    dA = (aA - aR).astype(np.float64)
    dA = np.where(dA == 0, 1e-30, dA)
    out[:, 12] = ((1.0 - aR.astype(np.float64)) / dA).astype(f32)  # dstar
    out_lR = lR.astype(f32)
    return out, out_lR


def build_program(L):
    """Build the SPMD Bass program for chunk length L (8192 for the real
    problem). Returns the compiled Bacc."""
    nc = bacc.Bacc("TRN2", target_bir_lowering=False, debug=False,
                   num_devices=N_CORES)
    x_in = nc.dram_tensor("x", (P, L), dt.float32, kind="ExternalInput")
    cst_in = nc.dram_tensor("cst", (P, NCONST), dt.float32, kind="ExternalInput")
    aux_in = nc.dram_tensor("aux", (4, P), dt.float32, kind="ExternalInput")
    ident_in = nc.dram_tensor("ident", (P, P), dt.float32, kind="ExternalInput")
    y_out = nc.dram_tensor("y", (P, L), dt.float32, kind="ExternalOutput")

    v = nc.vector
    s = nc.scalar
    g = nc.gpsimd
    te = nc.tensor

    NB = 4                      # col blocks for iter/post pipelining
    LB = L // NB
    NBP = 8                     # pre col blocks
    LBP = L // NBP

    with TileContext(nc) as tc:
        with (
            tc.tile_pool(name="big", bufs=1) as big,
            tc.tile_pool(name="sm", bufs=2) as sm,
            tc.tile_pool(name="smk", bufs=4) as smk,
            tc.tile_pool(name="ps", bufs=1, space="PSUM") as ps,
            tc.tile_pool(name="dram", bufs=1, space="DRAM") as dram,
        ):
            # ---- persistent small tiles
            cst = sm.tile([P, NCONST], dt.float32, tag="cst")
            nc.sync.dma_start(out=cst[:], in_=cst_in[:])
            maskt = sm.tile([1, P], dt.float32, tag="maskt")
            nc.sync.dma_start(out=maskt[:], in_=aux_in[0:1, :])
            mtt = sm.tile([2, P], dt.float32, tag="mtt")
            nc.sync.dma_start(out=mtt[:], in_=aux_in[1:3, :])
            onest = sm.tile([1, 1], dt.float32, tag="onest")
            nc.sync.dma_start(out=onest[:], in_=aux_in[3:4, 0:1])
            ident = sm.tile([P, P], dt.float32, tag="ident")
            nc.sync.dma_start(out=ident[:], in_=ident_in[:])
            startmask = maskt[0:1, :]    # [1,128]: 0 at chunk 0 and 64
            mt = mtt[0:2, :]             # [2,128] row-block indicator
            ones11 = onest[0:1, 0:1]     # [1,1] = 1.0

            def col(i):
                return cst[:, i:i + 1]

            # ---- big slots S1..S5 (32KB/partition each)
            S1 = big.tile([P, L], dt.float32, tag="S1")  # ND
            S2 = big.tile([P, L], dt.float32, tag="S2")  # x -> temps -> D
            S3 = big.tile([P, L], dt.float32, tag="S3")  # XL -> (spill) Vtmp
            S4 = big.tile([P, L], dt.float32, tag="S4")  # V
            S5 = big.tile([P, L], dt.float32, tag="S5")  # d/a
            ma32 = big.tile([P, L], dt.int32, tag="S4")  # PRE-only alias of S4
            spill = dram.tile([P, L], dt.float32, tag="spill")

            # ================= PRE: x -> x_L, D, ND (col-blocked) ========
            for b in range(NBP):
                sl = slice(b * LBP, (b + 1) * LBP)
                nc.sync.dma_start(out=S2[:, sl], in_=x_in[:, sl])
                # ACT: abs -> ln -> (CL*ln + (W-T))^2 -> knee; DVE off ln
                s.activation(S1[:, sl], S2[:, sl], Act.Abs, bias=0.0, scale=1.0)
                s.activation(S2[:, sl], S1[:, sl], Act.Ln, bias=col(11), scale=1.0)
                s.activation(S1[:, sl], S2[:, sl], Act.Square, bias=col(5),
                             scale=float(CL))
                s.activation(S3[:, sl], S1[:, sl], Act.Identity, bias=0.0,
                             scale=col(2))
                v.tensor_scalar(out=S5[:, sl], in0=S2[:, sl], scalar1=col(1),
                                scalar2=col(0), op0=Op.mult, op1=Op.add)
                v.tensor_scalar(out=ma32[:, sl], in0=S2[:, sl], scalar1=col(3),
                                scalar2=None, op0=Op.is_gt)
                v.copy_predicated(S3[:, sl], ma32[:, sl], S5[:, sl])
                v.tensor_scalar(out=S5[:, sl], in0=S2[:, sl], scalar1=col(4),
                                scalar2=None, op0=Op.is_ge)
                v.tensor_tensor(out=S3[:, sl], in0=S3[:, sl], in1=S5[:, sl],
                                op=Op.mult)
                # S3[:, sl] = x_L block. delta into S2 (cols shifted by 1)
                lo = b * LBP
                hi = (b + 1) * LBP
                v.tensor_tensor(out=S2[:, max(lo, 1):hi],
                                in0=S3[:, max(lo, 1) - 1:hi - 1],
                                in1=S3[:, max(lo, 1):hi], op=Op.subtract)
                s.activation(S1[:, max(lo, 1):hi], S2[:, max(lo, 1):hi],
                             Act.Identity, bias=0.0, scale=-1.0)
                nc.sync.dma_start(out=spill[:, sl], in_=S3[:, sl])
            # cross-chunk delta col 0: prevlast[p] = x_L[p-1, L-1], rows reset 0
            pl = smk.tile([P, 1], dt.float32, tag="pl")
            v.memset(pl[:], 0.0)
            nc.sync.dma_start(out=pl[1:P, :], in_=S3[0:P - 1, L - 1:L])
            v.memset(pl[64:65, :], 0.0)
            v.memset(pl[0:1, :], 0.0)
            v.tensor_tensor(out=S2[:, 0:1], in0=pl[:], in1=S3[:, 0:1],
                            op=Op.subtract)
            v.tensor_scalar(out=S1[:, 0:1], in0=S2[:, 0:1], scalar1=-1.0,
                            scalar2=None, op0=Op.mult)

            # ================= iteration machinery =================
            def boundary_chain(V_t, vinit_used, sd, bias_ap):
                """next v_init col from this sweep's (A, f).
                logA = dal*sd + bias_ap (bias holds lRL [+ dal*d0])."""
                logA = smk.tile([P, 1], dt.float32, tag="logA")
                v.scalar_tensor_tensor(out=logA[:], in0=sd, scalar=col(9),
                                       in1=bias_ap, op0=Op.mult, op1=Op.add)
                A_c = smk.tile([P, 1], dt.float32, tag="A_c")
                s.activation(A_c[:], logA[:], Act.Exp, bias=0.0, scale=1.0)
                t1 = smk.tile([P, 1], dt.float32, tag="t1")
                if vinit_used is None:
                    v.memset(t1[:], 0.0)
                else:
                    v.tensor_tensor(out=t1[:], in0=A_c[:], in1=vinit_used,
                                    op=Op.mult)
                f_c = smk.tile([P, 1], dt.float32, tag="f_c")
                v.tensor_tensor(out=f_c[:], in0=V_t[:, L - 1:L], in1=t1[:],
                                op=Op.subtract)
                ap_p = ps.tile([1, P], dt.float32, tag="ap_p")
                te.transpose(ap_p[:], A_c[:], ident[:])
                a_row = smk.tile([1, P], dt.float32, tag="a_row")
                v.tensor_tensor(out=a_row[:], in0=ap_p[:], in1=startmask,
                                op=Op.mult)
                fp_p = ps.tile([1, P], dt.float32, tag="fp_p")
                te.transpose(fp_p[:], f_c[:], ident[:])
                f_row = smk.tile([1, P], dt.float32, tag="f_row")
                v.tensor_copy(f_row[:], fp_p[:])
                zr = smk.tile([1, P], dt.float32, tag="zr")
                v.tensor_tensor_scan(zr[:], a_row[:], f_row[:], 0.0,
                                     Op.mult, Op.add)
                zs = smk.tile([1, P], dt.float32, tag="zs")
                v.memset(zs[:], 0.0)
                v.tensor_copy(zs[0:1, 1:P], zr[0:1, 0:P - 1])
                v.tensor_tensor(out=zs[:], in0=zs[:], in1=startmask, op=Op.mult)
                vip = ps.tile([P, 1], dt.float32, tag="vip")
                te.transpose(vip[:], zs[:], ones11)
                vic = smk.tile([P, 1], dt.float32, tag="vic")
                v.tensor_copy(vic[:], vip[:])
                return vic, zs

            def row_broadcast(pair_row):
                pr = ps.tile([2, 1], dt.float32, tag="pr")
                te.transpose(pr[:], pair_row, ones11)
                prs = smk.tile([2, 1], dt.float32, tag="prs")
                v.tensor_copy(prs[:], pr[:])
                cb = ps.tile([P, 1], dt.float32, tag="cb")
                te.matmul(cb[:], mt, prs[:])
                out = smk.tile([P, 1], dt.float32, tag="bc")
                v.tensor_copy(out[:], cb[:])
                return out

            vinit = None
            vinit_prev = None
            zrow_hist = []
            V_cur = None

            for k, step in enumerate(SCHED):
                sd = smk.tile([P, 1], dt.float32, tag="sd")
                lbias = col(8)  # lRL
                if k == 0:
                    v.tensor_scalar(out=S5[:], in0=S2[:], scalar1=0.0,
                                    scalar2=None, op0=Op.is_lt)
                    v.tensor_copy(S5[0:1, 0:1], cst[0:1, 12:13])
                    v.tensor_copy(S5[64:65, 0:1], cst[64:65, 12:13])
                    s.activation(S5[:], S5[:], Act.Identity, bias=0.0,
                                 scale=1.0, accum_out=sd[:])
                    vinit_used = None
                else:
                    Vp = V_cur
                    vic_used = vinit
                    if step == "E":
                        z0, z1, z2 = zrow_hist[-1], zrow_hist[-2], zrow_hist[-3]
                        d1 = smk.tile([1, P], dt.float32, tag="d1")
                        v.tensor_tensor(out=d1[:], in0=z0[:], in1=z1[:], op=Op.subtract)
                        s.activation(d1[:], d1[:], Act.Abs, bias=0.0, scale=1.0)
                        d2 = smk.tile([1, P], dt.float32, tag="d2")
                        v.tensor_tensor(out=d2[:], in0=z1[:], in1=z2[:], op=Op.subtract)
                        s.activation(d2[:], d2[:], Act.Abs, bias=0.0, scale=1.0)
                        rs = smk.tile([1, 2], dt.float32, tag="rs")
                        rs2 = smk.tile([1, 2], dt.float32, tag="rs2")
                        half = P // 2
                        v.tensor_reduce(rs[0:1, 0:1], d1[0:1, 0:half], mybir.AxisListType.X, Op.add)
                        v.tensor_reduce(rs[0:1, 1:2], d1[0:1, half:P], mybir.AxisListType.X, Op.add)
                        v.tensor_reduce(rs2[0:1, 0:1], d2[0:1, 0:half], mybir.AxisListType.X, Op.add)
                        v.tensor_reduce(rs2[0:1, 1:2], d2[0:1, half:P], mybir.AxisListType.X, Op.add)
                        v.tensor_scalar(out=rs2[:], in0=rs2[:], scalar1=1e-30,
                                        scalar2=None, op0=Op.add)
                        rho = smk.tile([1, 2], dt.float32, tag="rho")
                        v.reciprocal(rs2[:], rs2[:])
                        v.tensor_tensor(out=rho[:], in0=rs[:], in1=rs2[:], op=Op.mult)
                        v.tensor_scalar(out=rho[:], in0=rho[:], scalar1=0.95,
                                        scalar2=None, op0=Op.min)
                        om = smk.tile([1, 2], dt.float32, tag="om")
                        v.tensor_scalar(out=om[:], in0=rho[:], scalar1=-1.0,
                                        scalar2=1.0, op0=Op.mult, op1=Op.add)
                        v.reciprocal(om[:], om[:])
                        gam = smk.tile([1, 2], dt.float32, tag="gam")
                        v.tensor_tensor(out=gam[:], in0=rho[:], in1=om[:], op=Op.mult)
                        gcol = row_broadcast(gam[:])
                        gp1 = smk.tile([P, 1], dt.float32, tag="gp1")
                        v.tensor_scalar(out=gp1[:], in0=gcol[:], scalar1=1.0,
                                        scalar2=None, op0=Op.add)
                        Vm1 = S4 if V_cur is S3 else S3
                        v.tensor_scalar(out=Vm1[:], in0=Vm1[:], scalar1=gcol[:],
                                        scalar2=None, op0=Op.mult)
                        v.scalar_tensor_tensor(out=Vm1[:], in0=Vp[:],
                                               scalar=gp1[:], in1=Vm1[:],
                                               op0=Op.mult, op1=Op.subtract)
                        Vp = Vm1
                        dv = smk.tile([P, 1], dt.float32, tag="dv")
                        v.tensor_tensor(out=dv[:], in0=vinit[:], in1=vinit_prev[:],
                                        op=Op.subtract)
                        vice = smk.tile([P, 1], dt.float32, tag="vice")
                        v.scalar_tensor_tensor(out=vice[:], in0=dv[:], scalar=gcol[:],
                                               in1=vinit[:], op0=Op.mult, op1=Op.add)
                        vic_used = vice
                    # blocked d with per-block partial sums (stt bypass/is_gt)
                    sdb = smk.tile([P, NB], dt.float32, tag="sdb")
                    v.tensor_tensor(out=S5[:, 0:1], in0=vic_used[:], in1=S2[:, 0:1],
                                    op=Op.is_gt)
                    for b in range(NB):
                        lo = b * LB
                        hi = (b + 1) * LB
                        l2 = max(lo, 1)
                        v.scalar_tensor_tensor(out=S5[:, l2:hi],
                                               in0=Vp[:, l2 - 1:hi - 1],
                                               scalar=1.0, in1=S2[:, l2:hi],
                                               op0=Op.bypass, op1=Op.is_gt,
                                               accum_out=sdb[:, b:b + 1])
                    v.tensor_reduce(sd[:], sdb[:], mybir.AxisListType.X, Op.add)
                    # fold col-0 decision into the logA bias
                    tl = smk.tile([P, 1], dt.float32, tag="tl")
                    v.tensor_scalar(out=tl[:], in0=S5[:, 0:1], scalar1=col(9),
                                    scalar2=col(8), op0=Op.mult, op1=Op.add)
                    lbias = tl[:]
                    vinit_used = vic_used
                # a = dA*d + aR (ACT, hidden under the scan chain), then
                # row-start a[.,0]=1, and chained block scans
                Vout = S3 if k == 2 else S4
                a_dst = S3 if k == 0 else S5
                if k > 0:
                    v.tensor_copy(S5[0:1, 0:1], cst[0:1, 12:13])
                    v.tensor_copy(S5[64:65, 0:1], cst[64:65, 12:13])
                for b in range(NB):
                    lo = b * LB
                    hi = (b + 1) * LB
                    if b == 0:
                        v.tensor_scalar(out=a_dst[:, lo:hi], in0=S5[:, lo:hi],
                                        scalar1=col(7), scalar2=col(6),
                                        op0=Op.mult, op1=Op.add)
                    else:
                        s.activation(a_dst[:, lo:hi], S5[:, lo:hi], Act.Identity,
                                     bias=col(6), scale=col(7))
                    if b == 0:
                        init_ap = 0.0 if vinit_used is None else vinit_used[:]
                    else:
                        init_ap = Vout[:, lo - 1:lo]
                    v.tensor_tensor_scan(Vout[:, lo:hi], S1[:, lo:hi],
                                         a_dst[:, lo:hi], init_ap,
                                         Op.add, Op.mult)
                V_cur = Vout
                vinit_prev = vinit
                vic, zs = boundary_chain(
                    Vout, None if vinit_used is None else vinit_used[:], sd[:],
                    lbias)
                vinit = vic
                zrow_hist.append(zs)
                if len(zrow_hist) > 3:
                    zrow_hist.pop(0)

            # final exact re-scan (blocked) with POST chained per block
            nc.sync.dma_start(out=S3[:], in_=spill[:])
            nc.sync.dma_start(out=S2[:], in_=x_in[:])
            for b in range(NB):
                lo = b * LB
                hi = (b + 1) * LB
                sl = slice(lo, hi)
                init_ap = vinit[:] if b == 0 else S4[:, lo - 1:lo]
                v.tensor_tensor_scan(S4[:, sl], S1[:, sl], S5[:, sl], init_ap,
                                     Op.add, Op.mult)
                v.tensor_tensor(out=S5[:, sl], in0=S4[:, sl], in1=S3[:, sl],
                                op=Op.subtract)
                v.tensor_scalar(out=S5[:, sl], in0=S5[:, sl], scalar1=96.0,
                                scalar2=None, op0=Op.min)
                s.activation(S1[:, sl], S5[:, sl], Act.Exp, bias=col(10),
                             scale=float(RGAIN))
                v.tensor_tensor(out=S3[:, sl], in0=S1[:, sl], in1=S2[:, sl],
                                op=Op.mult)
                nc.sync.dma_start(out=y_out[:, sl], in_=S3[:, sl])

    nc.compile()
    return nc


_CACHE = {}
PROFILE = False
LAST_EXEC_NS = None
LAST_RESULTS = None


def _get_program(L):
    if L not in _CACHE:
        _CACHE[L] = build_program(L)
    return _CACHE[L]


def make_core_inputs(x, params, L):
    """Full x [B,N], params [B,6] -> list of per-core input dicts."""
    B, N = x.shape
    n_cores = B // ROWS_PER_CORE
    rows_per_core = ROWS_PER_CORE
    csts, lR = host_consts(params)
    csts[:, 8] = (lR.astype(np.float64) * L).astype(f32)
    aux = np.zeros((4, P), f32)
    aux[0, :] = 1.0
    aux[0, 0] = 0.0
    aux[0, 64] = 0.0
    aux[1, 0:64] = 1.0   # Mt row 0 -> partitions 0..63
    aux[2, 64:128] = 1.0
    aux[3, 0] = 1.0      # ones11
    ident = np.eye(P, dtype=f32)
    in_maps = []
    for c in range(n_cores):
        rows = slice(c * rows_per_core, (c + 1) * rows_per_core)
        xs = np.ascontiguousarray(x[rows]).reshape(P, L)
        cc = np.repeat(csts[rows], P // rows_per_core, axis=0)
        in_maps.append(dict(x=xs, cst=np.ascontiguousarray(cc),
                            aux=aux, ident=ident))
    return in_maps


def kernel(x, params):
    x = np.asarray(x)
    params = np.asarray(params, f32)
    B, N = x.shape
    L = (N * ROWS_PER_CORE) // P
    nc = _get_program(L)
    in_maps = make_core_inputs(np.asarray(x, f32), params, L)
    global LAST_EXEC_NS, LAST_RESULTS
    res = run_bass_kernel_spmd(nc, in_maps, list(range(B // ROWS_PER_CORE)),
                               trace=PROFILE)
    LAST_EXEC_NS = res.exec_time_ns
    LAST_RESULTS = res
    outs = [r["y"].reshape(ROWS_PER_CORE, N) for r in res.results]
    return np.concatenate(outs, axis=0).astype(x.dtype, copy=False)
